# revision 8
# baseline (speedup 1.0000x reference)
"""DenseGATConv-style GNN message passing kernel for Trainium2 (Bass/Tile).

Math (per graph b):
    e      = w_edge[edge_attr[b]]            # [N, N] gather from 4-entry table
    adj_w  = adj[b] * e                      # weighted adjacency
    agg    = adj_w @ x[b]                    # [N, C]
    out[b] = agg @ W_rel + b_rel + x[b] @ W_root

Key trick: the 4-entry gather w_edge[a], a in {0,1,2,3}, equals the cubic
polynomial through the 4 points, evaluated in factored form
    p(a) = c3 * (a - r) * ((a + h)^2 + v2)
so adj_w/c3 is computed with 2 fused scalar_tensor_tensor ops + 1 activation
(or 3 STT ops), and c3 is folded into W_rel on the host (weight folding).

Sharding: data-parallel over batch B=16 across 8 cores (2 graphs/core);
weights replicated.
"""

import sys
from contextlib import ExitStack

sys.path.insert(0, "/opt/trn_rl_repo")

import numpy as np

_B, _N, _C = 16, 1024, 64
_NCORES = 8
_G = _B // _NCORES  # graphs per core
_P = 128
_NT = _N // _P  # 128-row tiles per graph

# Module-level knobs (test.py may flip these before calling kernel()).
TRACE = False
# "pool_stt": square-ish op (a+p)*a on GPSIMD as scalar_tensor_tensor
# "act_sq":   square op (a+h)^2 on ScalarE as Activation(Square)
# "dve_stt":  everything on the vector engine
SQUARE_ENGINE = "act_sq"
LAST_RESULTS = None  # BassKernelResults of the most recent run (for test.py)

_BUILD_CACHE = {}


def _poly_coeffs(w_edge):
    """Cubic through (k, w_edge[k]) for k=0..3, float64. Returns c0..c3."""
    w = np.asarray(w_edge, dtype=np.float64).reshape(4)
    V = np.vander(np.arange(4.0), 4, increasing=True)
    c = np.linalg.solve(V, w)
    return c  # [c0, c1, c2, c3]


def _chain_params(w_edge):
    """Pick the elementwise chain and host-folded scale from w_edge values.

    Returns (mode, params, lead) where `lead` multiplies W_rel on the host and
    the device computes adj_w/lead.
    """
    c0, c1, c2, c3 = _poly_coeffs(w_edge)
    scale = max(np.max(np.abs(np.asarray(w_edge, dtype=np.float64))), 1e-30)
    tol = 1e-7 * scale
    if abs(c3) > tol:
        # monic cubic a^3 + A a^2 + B a + C = (a - r)(a^2 + p a + q)
        A, Bc, Cc = c2 / c3, c1 / c3, c0 / c3
        roots = np.roots([1.0, A, Bc, Cc])
        r = float(np.real(roots[np.argmin(np.abs(np.imag(roots)))]))
        p = A + r
        q = Bc + p * r
        # completed square for the ACT path: a^2 + p a + q = (a + p/2)^2 + v2
        return "cubic", dict(r=r, p=p, q=q, h=p / 2.0, v2=q - p * p / 4.0), c3
    if abs(c2) > tol:
        p2, q2 = c1 / c2, c0 / c2
        return "quad", dict(p=p2, q=q2, h=p2 / 2.0, v2=q2 - p2 * p2 / 4.0), c2
    if abs(c1) > tol:
        return "linear", dict(r=-c0 / c1), c1
    return "const", dict(), c0


def _emit_square(nc, OP, AF, s_out, ea_ap, params, square_engine, pools=None):
    """s_out <- quadratic-part tensor; returns the constant to add to it."""
    if square_engine == "act_sq":
        nc.scalar.activation(
            s_out, ea_ap, AF.Square, bias=pools["hbias_sb"][:, 0:1], scale=1.0
        )
        return float(params["v2"])
    eng = nc.gpsimd if square_engine == "pool_stt" else nc.vector
    eng.scalar_tensor_tensor(
        s_out, ea_ap, float(-params["p"]), ea_ap, OP.subtract, OP.mult
    )
    return float(params["q"])


def _emit_elementwise(nc, OP, AF, pools, ea_t, adj_t, mode, params, square_engine):
    """Emit adj_w/lead for one [128, N] tile; returns the aw tile."""
    sp, qtp, awp = pools["sp"], pools["qtp"], pools["awp"]
    if mode == "cubic":
        qt_t = qtp.tile([_P, _N], pools["f32"])
        nc.vector.scalar_tensor_tensor(
            qt_t[:], ea_t[:], float(params["r"]), adj_t[:], OP.subtract, OP.mult
        )
        s_t = sp.tile([_P, _N], pools["f32"])
        k_add = _emit_square(nc, OP, AF, s_t[:], ea_t[:], params, square_engine, pools)
        aw_t = awp.tile([_P, _N], pools["f32"])
        nc.vector.scalar_tensor_tensor(
            aw_t[:], s_t[:], k_add, qt_t[:], OP.add, OP.mult
        )
        return aw_t
    if mode == "quad":
        s_t = sp.tile([_P, _N], pools["f32"])
        k_add = _emit_square(nc, OP, AF, s_t[:], ea_t[:], params, square_engine, pools)
        aw_t = awp.tile([_P, _N], pools["f32"])
        nc.vector.scalar_tensor_tensor(
            aw_t[:], s_t[:], k_add, adj_t[:], OP.add, OP.mult
        )
        return aw_t
    if mode == "linear":
        aw_t = awp.tile([_P, _N], pools["f32"])
        nc.vector.scalar_tensor_tensor(
            aw_t[:], ea_t[:], float(params["r"]), adj_t[:], OP.subtract, OP.mult
        )
        return aw_t
    return adj_t  # const


def _emit_graph(nc, tc, pools, g, dram, mode, params, square_engine):
    from concourse import mybir

    OP = mybir.AluOpType
    AF = mybir.ActivationFunctionType
    f32 = pools["f32"]
    x_d, adj_d, ea_d, out_d = dram["x"], dram["adj"], dram["ea"], dram["out"]
    ident = pools["ident"]

    # x in aggregation layout: xs[p, t*C+c] = x[t*128+p, c]
    xs = pools["xsp"].tile([_P, _NT * _C], f32)
    nc.sync.dma_start(
        out=xs[:].rearrange("p (t c) -> p t c", t=_NT),
        in_=x_d[g, :, :].rearrange("(t p) c -> p t c", p=_P),
    )
    # x^T [c, i] for the root-term matmul
    p_xT = pools["ps_xa"].tile([_C, _N], f32, tag="ps_xa")
    for jt in range(_NT):
        nc.tensor.transpose(
            p_xT[:, jt * _P : (jt + 1) * _P],
            xs[:, jt * _C : (jt + 1) * _C],
            ident[:],
        )
    xT = pools["xTp"].tile([_C, _N], f32)
    nc.scalar.copy(out=xT[:], in_=p_xT[:])

    # aggregation accumulator: agg[p, it*C+c] = (adj_w @ x)[it*128+p, c]
    p_agg = pools["ps_agg"].tile([_P, _NT * _C], f32, tag="ps_agg")

    for it in range(_NT):
        adj_t = pools["adjp"].tile([_P, _N], f32)
        nc.sync.dma_start(out=adj_t[:], in_=adj_d[g, it * _P : (it + 1) * _P, :])
        ea_t = pools["eap"].tile([_P, _N], pools["i32"])
        nc.sync.dma_start(out=ea_t[:], in_=ea_d[g, it * _P : (it + 1) * _P, :])

        aw_t = _emit_elementwise(
            nc, OP, AF, pools, ea_t, adj_t, mode, params, square_engine
        )

        # transpose adj_w tile blockwise (4 blocks per PSUM bank), then
        # accumulate agg[it-block] over all j
        for half in range(2):
            p_tp = pools["ps_tp"].tile([_P, 4 * _P], f32, tag="ps_tp")
            for k in range(4):
                jt = half * 4 + k
                nc.tensor.transpose(
                    p_tp[:, k * _P : (k + 1) * _P],
                    aw_t[:, jt * _P : (jt + 1) * _P],
                    ident[:],
                )
            awT = pools["awTp"].tile([_P, 4 * _P], f32)
            nc.scalar.copy(out=awT[:], in_=p_tp[:])
            for k in range(4):
                jt = half * 4 + k
                nc.tensor.matmul(
                    p_agg[:, it * _C : (it + 1) * _C],
                    lhsT=awT[:, k * _P : (k + 1) * _P],
                    rhs=xs[:, jt * _C : (jt + 1) * _C],
                    start=(jt == 0),
                    stop=(jt == _NT - 1),
                )

    agg_sb = pools["aggp"].tile([_P, _NT * _C], f32)
    nc.scalar.copy(out=agg_sb[:], in_=p_agg[:])

    # transpose agg -> agg^T [c, i]
    p_aggT = pools["ps_xa"].tile([_C, _N], f32, tag="ps_xa")
    for it in range(_NT):
        nc.tensor.transpose(
            p_aggT[:, it * _P : (it + 1) * _P],
            agg_sb[:, it * _C : (it + 1) * _C],
            ident[:],
        )
    aggT = pools["aggTp"].tile([_C, _N], f32)
    nc.scalar.copy(out=aggT[:], in_=p_aggT[:])

    # out^T[c', i] = W_rel-contraction + W_root-term
    p_out = pools["ps_o"].tile([_C, _N], f32, tag="ps_o")
    for h in range(2):
        sl = slice(h * 512, (h + 1) * 512)
        nc.tensor.matmul(
            p_out[:, sl], lhsT=pools["wrel_sb"][:], rhs=aggT[:, sl],
            start=True, stop=False,
        )
        nc.tensor.matmul(
            p_out[:, sl], lhsT=pools["wroot_sb"][:], rhs=xT[:, sl],
            start=False, stop=True,
        )
    outT = pools["outTp"].tile([_C, _N], f32)
    nc.vector.tensor_scalar(
        outT[:], p_out[:], pools["brel_sb"][:, 0:1], None, OP.add
    )

    # transpose back to natural [i, c] layout and store
    p_on = pools["ps_o"].tile([_P, _NT * _C], f32, tag="ps_o")
    for it in range(_NT):
        nc.tensor.transpose(
            p_on[:, it * _C : (it + 1) * _C],
            outT[:, it * _P : (it + 1) * _P],
            ident[:_C, :_C],
        )
    out_sb = pools["outp"].tile([_P, _NT * _C], f32)
    nc.scalar.copy(out=out_sb[:], in_=p_on[:])
    nc.sync.dma_start(
        out=out_d[g, :, :].rearrange("(t p) c -> p t c", p=_P),
        in_=out_sb[:].rearrange("p (t c) -> p t c", t=_NT),
    )


def _build_module(mode, params, square_engine):
    import concourse.bass as bass  # noqa: F401
    from concourse import bacc, mybir
    from concourse.masks import make_identity
    from concourse.tile import TileContext

    f32 = mybir.dt.float32
    i32 = mybir.dt.int32

    nc = bacc.Bacc(
        "TRN2", target_bir_lowering=False, debug=False, num_devices=_NCORES
    )

    dram = {
        "x": nc.dram_tensor("x", [_G, _N, _C], f32, kind="ExternalInput"),
        "adj": nc.dram_tensor("adj", [_G, _N, _N], f32, kind="ExternalInput"),
        "ea": nc.dram_tensor("ea", [_G, _N, _N], i32, kind="ExternalInput"),
        "wrel": nc.dram_tensor("wrel", [_C, _C], f32, kind="ExternalInput"),
        "wroot": nc.dram_tensor("wroot", [_C, _C], f32, kind="ExternalInput"),
        "brel": nc.dram_tensor("brel", [_C, 1], f32, kind="ExternalInput"),
        "out": nc.dram_tensor("out", [_G, _N, _C], f32, kind="ExternalOutput"),
    }

    pool_specs = [
        ("consts", 1, None),
        ("adjp", 6, None),
        ("eap", 6, None),
        ("sp", 3, None),
        ("qtp", 3, None),
        ("awp", 3, None),
        ("awTp", 4, None),
        ("xsp", 2, None),
        ("xTp", 2, None),
        ("aggp", 2, None),
        ("aggTp", 2, None),
        ("outTp", 2, None),
        ("outp", 2, None),
        ("ps_tp", 2, "PSUM"),
        ("ps_agg", 2, "PSUM"),
        ("ps_xa", 1, "PSUM"),
        ("ps_o", 1, "PSUM"),
    ]

    with TileContext(nc) as tc, ExitStack() as ctx:
        pools = {"f32": f32, "i32": i32}
        for name, bufs, space in pool_specs:
            kw = {"space": space} if space else {}
            pools[name] = ctx.enter_context(tc.tile_pool(name=name, bufs=bufs, **kw))

        ident = pools["consts"].tile([_P, _P], f32)
        make_identity(nc, ident[:])
        pools["ident"] = ident
        for wname, shape in (("wrel", [_C, _C]), ("wroot", [_C, _C]), ("brel", [_C, 1])):
            t = pools["consts"].tile(shape, f32, tag=wname)
            nc.sync.dma_start(out=t[:], in_=dram[wname][:, :])
            pools[wname + "_sb"] = t

        if square_engine == "act_sq" and mode in ("cubic", "quad"):
            hb = pools["consts"].tile([_P, 1], f32)
            nc.vector.memset(hb[:], float(params["h"]))
            pools["hbias_sb"] = hb

        for g in range(_G):
            _emit_graph(nc, tc, pools, g, dram, mode, params, square_engine)

    nc.finalize()
    return nc


def _get_module(w_edge, square_engine):
    mode, params, lead = _chain_params(w_edge)
    key = (
        mode,
        tuple(sorted((k, round(v, 15)) for k, v in params.items())),
        square_engine,
    )
    if key not in _BUILD_CACHE:
        _BUILD_CACHE[key] = _build_module(mode, params, square_engine)
    return _BUILD_CACHE[key], lead


def _prep_inputs(x, adj, edge_attr, W_rel, b_rel, W_root, w_edge):
    x = np.ascontiguousarray(np.asarray(x, dtype=np.float32))
    adj = np.ascontiguousarray(np.asarray(adj, dtype=np.float32))
    ea = np.ascontiguousarray(np.asarray(edge_attr, dtype=np.int32).reshape(_B, _N, _N))
    W_rel = np.asarray(W_rel, dtype=np.float64)
    W_root = np.ascontiguousarray(np.asarray(W_root, dtype=np.float32))
    b_rel = np.ascontiguousarray(np.asarray(b_rel, dtype=np.float32).reshape(_C, 1))
    w_edge = np.asarray(w_edge)
    return x, adj, ea, W_rel, b_rel, W_root, w_edge


def kernel(x, adj, edge_attr, W_rel, b_rel, W_root, w_edge):
    global LAST_RESULTS
    from concourse.bass_utils import run_bass_kernel_spmd

    x, adj, ea, W_rel, b_rel, W_root, w_edge = _prep_inputs(
        x, adj, edge_attr, W_rel, b_rel, W_root, w_edge
    )
    nc, lead = _get_module(w_edge, SQUARE_ENGINE)
    wrel_eff = np.ascontiguousarray((lead * W_rel).astype(np.float32))

    in_maps = []
    for c in range(_NCORES):
        sl = slice(c * _G, (c + 1) * _G)
        in_maps.append(
            {
                "x": x[sl],
                "adj": adj[sl],
                "ea": ea[sl],
                "wrel": wrel_eff,
                "wroot": W_root,
                "brel": b_rel,
            }
        )

    res = run_bass_kernel_spmd(nc, in_maps, list(range(_NCORES)), trace=TRACE)
    LAST_RESULTS = res
    out = np.concatenate([res.results[c]["out"] for c in range(_NCORES)], axis=0)
    return out


# revision 11
# speedup vs baseline: 1.2699x; 1.2699x over previous
"""DenseGATConv-style GNN message passing kernel for Trainium2 (Bass/Tile).

Math (per graph b):
    e      = w_edge[edge_attr[b]]            # [N, N] gather from 4-entry table
    adj_w  = adj[b] * e                      # weighted adjacency
    agg    = adj_w @ x[b]                    # [N, C]
    out[b] = agg @ W_rel + b_rel + x[b] @ W_root

Key tricks:
  * The 4-entry gather w_edge[a], a in {0,1,2,3}, equals the cubic polynomial
    through the 4 points, evaluated in factored form
        p(a) = c3 * (a - r) * ((a + p) * a + q)
    so adj_w/c3 is computed with 3 fused scalar_tensor_tensor ops, and c3 is
    folded into W_rel on the host (weight folding).
  * b_rel is folded into the W_rel matmul as a 65th contraction row against a
    constant ones-row appended to agg^T.
  * The aggregation runs in transposed layout (out^T = Wrel^T@aggT + ...),
    with adj_w transposed on the PE in 128x128 blocks, 8 blocks batched per
    PSUM->SBUF copy.

Sharding: data-parallel over batch B=16 across 8 cores (2 graphs/core);
weights replicated.
"""

import sys
from contextlib import ExitStack

sys.path.insert(0, "/opt/trn_rl_repo")

import numpy as np

_B, _N, _C = 16, 1024, 64
_NCORES = 8
_G = _B // _NCORES  # graphs per core
_P = 128
_NT = _N // _P  # 128-row tiles per graph

# Module-level knobs (test.py may flip these before calling kernel()).
TRACE = False
# "pool_stt": square-ish op (a+p)*a on GPSIMD as scalar_tensor_tensor
# "act_sq":   square op (a+h)^2 on ScalarE as Activation(Square)
# "dve_stt":  everything on the vector engine
SQUARE_ENGINE = "pool_stt"
# "float32": exact, PE-bound (~4 cyc/row).  "float32r": TF32-class matmul
# precision (~1.5e-4 rel) but 1 cyc/row for the wide aggregation matmul.
MM_DTYPE = "float32r"
LAST_RESULTS = None  # BassKernelResults of the most recent run (for test.py)

_BUILD_CACHE = {}


def _poly_coeffs(w_edge):
    """Cubic through (k, w_edge[k]) for k=0..3, float64. Returns c0..c3."""
    w = np.asarray(w_edge, dtype=np.float64).reshape(4)
    V = np.vander(np.arange(4.0), 4, increasing=True)
    c = np.linalg.solve(V, w)
    return c  # [c0, c1, c2, c3]


def _chain_params(w_edge):
    """Pick the elementwise chain and host-folded scale from w_edge values.

    Returns (mode, params, lead) where `lead` multiplies W_rel on the host and
    the device computes adj_w/lead.
    """
    c0, c1, c2, c3 = _poly_coeffs(w_edge)
    scale = max(np.max(np.abs(np.asarray(w_edge, dtype=np.float64))), 1e-30)
    tol = 1e-7 * scale
    if abs(c3) > tol:
        # monic cubic a^3 + A a^2 + B a + C = (a - r)(a^2 + p a + q)
        A, Bc, Cc = c2 / c3, c1 / c3, c0 / c3
        roots = np.roots([1.0, A, Bc, Cc])
        r = float(np.real(roots[np.argmin(np.abs(np.imag(roots)))]))
        p = A + r
        q = Bc + p * r
        # completed square for the ACT path: a^2 + p a + q = (a + p/2)^2 + v2
        return "cubic", dict(r=r, p=p, q=q, h=p / 2.0, v2=q - p * p / 4.0), c3
    if abs(c2) > tol:
        p2, q2 = c1 / c2, c0 / c2
        return "quad", dict(p=p2, q=q2, h=p2 / 2.0, v2=q2 - p2 * p2 / 4.0), c2
    if abs(c1) > tol:
        return "linear", dict(r=-c0 / c1), c1
    return "const", dict(), c0


def _emit_square(nc, OP, AF, s_out, ea_ap, params, square_engine, pools):
    """s_out <- quadratic-part tensor; returns the constant to add to it."""
    if square_engine == "act_sq":
        nc.scalar.activation(
            s_out, ea_ap, AF.Square, bias=pools["hbias_sb"][:, 0:1], scale=1.0
        )
        return float(params["v2"])
    eng = nc.gpsimd if square_engine == "pool_stt" else nc.vector
    eng.scalar_tensor_tensor(
        s_out, ea_ap, float(-params["p"]), ea_ap, OP.subtract, OP.mult
    )
    return float(params["q"])


def _emit_elementwise(nc, OP, AF, pools, ea_t, adj_t, mode, params, square_engine):
    """Emit adj_w/lead for one [128, N] tile slice pair; returns the aw AP."""
    sp, qtp, awp = pools["sp"], pools["qtp"], pools["awp"]
    mmdt = pools["mmdt"]
    f32 = pools["f32"]
    if mode == "cubic":
        qt_t = qtp.tile([_P, _N], f32)
        nc.vector.scalar_tensor_tensor(
            qt_t[:], ea_t, float(params["r"]), adj_t, OP.subtract, OP.mult
        )
        s_t = sp.tile([_P, _N], f32)
        k_add = _emit_square(nc, OP, AF, s_t[:], ea_t, params, square_engine, pools)
        aw_t = awp.tile([_P, _N], mmdt)
        nc.vector.scalar_tensor_tensor(
            aw_t[:], s_t[:], k_add, qt_t[:], OP.add, OP.mult
        )
        return aw_t
    if mode == "quad":
        s_t = sp.tile([_P, _N], f32)
        k_add = _emit_square(nc, OP, AF, s_t[:], ea_t, params, square_engine, pools)
        aw_t = awp.tile([_P, _N], mmdt)
        nc.vector.scalar_tensor_tensor(
            aw_t[:], s_t[:], k_add, adj_t, OP.add, OP.mult
        )
        return aw_t
    if mode == "linear":
        aw_t = awp.tile([_P, _N], mmdt)
        nc.vector.scalar_tensor_tensor(
            aw_t[:], ea_t, float(params["r"]), adj_t, OP.subtract, OP.mult
        )
        return aw_t
    # const: plain copy into the matmul dtype
    aw_t = awp.tile([_P, _N], mmdt)
    nc.vector.tensor_copy(aw_t[:], adj_t)
    return aw_t


def _emit_graph(nc, tc, pools, g, dram, mode, params, square_engine):
    from concourse import mybir

    OP = mybir.AluOpType
    AF = mybir.ActivationFunctionType
    f32 = pools["f32"]
    mmdt = pools["mmdt"]
    x_d, adj_d, ea_d, out_d = dram["x"], dram["adj"], dram["ea"], dram["out"]
    ident = pools["ident"]
    ident_m = pools["ident_m"]

    # x in aggregation layout: xs[p, t*C+c] = x[t*128+p, c]
    xs = pools["xsp"].tile([_P, _NT * _C], f32)
    nc.sync.dma_start(
        out=xs[:].rearrange("p (t c) -> p t c", t=_NT),
        in_=x_d[g, :, :].rearrange("(t p) c -> p t c", p=_P),
    )
    if mmdt is f32:
        xs_mm = xs
    else:
        xs_mm = pools["xsp"].tile([_P, _NT * _C], mmdt, tag="xs_mm")
        nc.vector.tensor_copy(xs_mm[:], xs[:])
    # x^T [c, i] for the root-term matmul (exact fp32 path)
    p_xT = pools["ps_misc"].tile([_C, _N], f32, tag="ps_misc")
    for jt in range(_NT):
        nc.tensor.transpose(
            p_xT[:, jt * _P : (jt + 1) * _P],
            xs[:, jt * _C : (jt + 1) * _C],
            ident[:],
        )
    xT = pools["xTp"].tile([_C, _N], f32)
    nc.scalar.copy(out=xT[:], in_=p_xT[:])

    # aggT accumulators: aggT[c, i] = (adj_w @ x)^T, one [64, 512] psum per
    # half-graph of i; row 64 of the SBUF copy is a constant 1.0 row for the
    # bias fold.
    aggT = pools["aggTp"].tile([_C + 1, _N], f32)
    nc.gpsimd.memset(aggT[_C : _C + 1, :], 1.0)

    for half in range(2):
        its = range(4 * half, 4 * half + 4)
        # DMA adj/ea for this half in two 1 MiB chunks of 2 tiles each,
        # then the elementwise chain per 128-row tile
        aw_list = []
        for pair in range(2):
            base = 4 * half + 2 * pair
            adj_t = pools["adjp"].tile([_P, 2 * _N], f32)
            nc.sync.dma_start(
                out=adj_t[:].rearrange("p (q j) -> p q j", q=2),
                in_=adj_d[g, base * _P : (base + 2) * _P, :].rearrange(
                    "(q p) j -> p q j", p=_P
                ),
            )
            ea_t = pools["eap"].tile([_P, 2 * _N], pools["i32"])
            nc.sync.dma_start(
                out=ea_t[:].rearrange("p (q j) -> p q j", q=2),
                in_=ea_d[g, base * _P : (base + 2) * _P, :].rearrange(
                    "(q p) j -> p q j", p=_P
                ),
            )
            for q in range(2):
                aw_list.append(
                    _emit_elementwise(
                        nc, OP, AF, pools,
                        ea_t[:, q * _N : (q + 1) * _N],
                        adj_t[:, q * _N : (q + 1) * _N],
                        mode, params, square_engine,
                    )
                )

        p_aggT = pools["ps_agg"].tile([_C, 512], f32, tag="ps_agg")
        for jtp in range(4):  # jt pairs
            p_tp = pools["ps_tp"].tile([_P, 2 * 512], mmdt, tag="ps_tp")
            for sub in range(2):
                jt = 2 * jtp + sub
                for k in range(4):
                    nc.tensor.transpose(
                        p_tp[:, sub * 512 + k * _P : sub * 512 + (k + 1) * _P],
                        aw_list[k][:, jt * _P : (jt + 1) * _P],
                        ident_m[:],
                    )
            awT = pools["awTp"].tile([_P, 2 * 512], mmdt)
            nc.scalar.copy(out=awT[:], in_=p_tp[:])
            for sub in range(2):
                jt = 2 * jtp + sub
                nc.tensor.matmul(
                    p_aggT[:],
                    lhsT=xs_mm[:, jt * _C : (jt + 1) * _C],
                    rhs=awT[:, sub * 512 : (sub + 1) * 512],
                    start=(jt == 0),
                    stop=(jt == _NT - 1),
                )
        nc.scalar.copy(out=aggT[:_C, half * 512 : (half + 1) * 512], in_=p_aggT[:])

    # out^T[c', i] = [W_rel; b_rel]^T @ [aggT; 1] + W_root^T @ xT
    p_out = pools["ps_misc"].tile([_C, _N], f32, tag="ps_misc")
    for h in range(2):
        sl = slice(h * 512, (h + 1) * 512)
        nc.tensor.matmul(
            p_out[:, sl], lhsT=pools["wrel_sb"][:], rhs=aggT[:, sl],
            start=True, stop=False,
        )
        nc.tensor.matmul(
            p_out[:, sl], lhsT=pools["wroot_sb"][:], rhs=xT[:, sl],
            start=False, stop=True,
        )
    outT = pools["outTp"].tile([_C, _N], f32)
    nc.scalar.copy(out=outT[:], in_=p_out[:])

    # transpose back to natural [i, c] layout and store
    p_on = pools["ps_misc"].tile([_P, _NT * _C], f32, tag="ps_misc")
    for it in range(_NT):
        nc.tensor.transpose(
            p_on[:, it * _C : (it + 1) * _C],
            outT[:, it * _P : (it + 1) * _P],
            ident[:_C, :_C],
        )
    out_sb = pools["outp"].tile([_P, _NT * _C], f32)
    nc.scalar.copy(out=out_sb[:], in_=p_on[:])
    nc.sync.dma_start(
        out=out_d[g, :, :].rearrange("(t p) c -> p t c", p=_P),
        in_=out_sb[:].rearrange("p (t c) -> p t c", t=_NT),
    )


def _build_module(mode, params, square_engine, mm_dtype):
    import concourse.bass as bass  # noqa: F401
    from concourse import bacc, mybir
    from concourse.masks import make_identity
    from concourse.tile import TileContext

    f32 = mybir.dt.float32
    i32 = mybir.dt.int32
    mmdt = getattr(mybir.dt, mm_dtype)

    nc = bacc.Bacc(
        "TRN2", target_bir_lowering=False, debug=False, num_devices=_NCORES
    )

    dram = {
        "x": nc.dram_tensor("x", [_G, _N, _C], f32, kind="ExternalInput"),
        "adj": nc.dram_tensor("adj", [_G, _N, _N], f32, kind="ExternalInput"),
        "ea": nc.dram_tensor("ea", [_G, _N, _N], i32, kind="ExternalInput"),
        "wrel": nc.dram_tensor("wrel", [_C + 1, _C], f32, kind="ExternalInput"),
        "wroot": nc.dram_tensor("wroot", [_C, _C], f32, kind="ExternalInput"),
        "out": nc.dram_tensor("out", [_G, _N, _C], f32, kind="ExternalOutput"),
    }

    pool_specs = [
        ("consts", 1, None),
        ("adjp", 3, None),
        ("eap", 3, None),
        ("sp", 3, None),
        ("qtp", 3, None),
        ("awp", 6, None),
        ("awTp", 3, None),
        ("xsp", 2, None),
        ("xTp", 2, None),
        ("aggTp", 2, None),
        ("outTp", 2, None),
        ("outp", 2, None),
        ("ps_tp", 2, "PSUM"),
        ("ps_agg", 2, "PSUM"),
        ("ps_misc", 1, "PSUM"),
    ]

    with TileContext(nc) as tc, ExitStack() as ctx:
        pools = {"f32": f32, "i32": i32, "mmdt": mmdt}
        for name, bufs, space in pool_specs:
            kw = {"space": space} if space else {}
            pools[name] = ctx.enter_context(tc.tile_pool(name=name, bufs=bufs, **kw))

        ident = pools["consts"].tile([_P, _P], f32, tag="ident")
        make_identity(nc, ident[:])
        pools["ident"] = ident
        if mm_dtype == "float32":
            pools["ident_m"] = ident
        else:
            ident_m = pools["consts"].tile([_P, _P], mmdt, tag="ident_m")
            nc.vector.tensor_copy(ident_m[:], ident[:])
            pools["ident_m"] = ident_m
        for wname, shape in (("wrel", [_C + 1, _C]), ("wroot", [_C, _C])):
            t = pools["consts"].tile(shape, f32, tag=wname)
            nc.sync.dma_start(out=t[:], in_=dram[wname][:, :])
            pools[wname + "_sb"] = t

        if square_engine == "act_sq" and mode in ("cubic", "quad"):
            hb = pools["consts"].tile([_P, 1], f32, tag="hb")
            nc.vector.memset(hb[:], float(params["h"]))
            pools["hbias_sb"] = hb

        for g in range(_G):
            _emit_graph(nc, tc, pools, g, dram, mode, params, square_engine)

    nc.finalize()
    return nc


def _get_module(w_edge, square_engine, mm_dtype):
    mode, params, lead = _chain_params(w_edge)
    key = (
        mode,
        tuple(sorted((k, round(v, 15)) for k, v in params.items())),
        square_engine,
        mm_dtype,
    )
    if key not in _BUILD_CACHE:
        _BUILD_CACHE[key] = _build_module(mode, params, square_engine, mm_dtype)
    return _BUILD_CACHE[key], lead


def _prep_inputs(x, adj, edge_attr, W_rel, b_rel, W_root, w_edge):
    x = np.ascontiguousarray(np.asarray(x, dtype=np.float32))
    adj = np.ascontiguousarray(np.asarray(adj, dtype=np.float32))
    ea = np.ascontiguousarray(np.asarray(edge_attr, dtype=np.int32).reshape(_B, _N, _N))
    W_rel = np.asarray(W_rel, dtype=np.float64)
    W_root = np.ascontiguousarray(np.asarray(W_root, dtype=np.float32))
    b_rel = np.asarray(b_rel, dtype=np.float32).reshape(1, _C)
    w_edge = np.asarray(w_edge)
    return x, adj, ea, W_rel, b_rel, W_root, w_edge


def kernel(x, adj, edge_attr, W_rel, b_rel, W_root, w_edge):
    global LAST_RESULTS
    from concourse.bass_utils import run_bass_kernel_spmd

    x, adj, ea, W_rel, b_rel, W_root, w_edge = _prep_inputs(
        x, adj, edge_attr, W_rel, b_rel, W_root, w_edge
    )
    nc, lead = _get_module(w_edge, SQUARE_ENGINE, MM_DTYPE)
    # [W_rel * lead; b_rel] stacked: the 65th row contracts against the
    # constant ones-row appended to agg^T.
    wrel_eff = np.ascontiguousarray(
        np.concatenate([lead * W_rel, b_rel.astype(np.float64)], axis=0).astype(
            np.float32
        )
    )

    in_maps = []
    for c in range(_NCORES):
        sl = slice(c * _G, (c + 1) * _G)
        in_maps.append(
            {
                "x": x[sl],
                "adj": adj[sl],
                "ea": ea[sl],
                "wrel": wrel_eff,
                "wroot": W_root,
            }
        )

    res = run_bass_kernel_spmd(nc, in_maps, list(range(_NCORES)), trace=TRACE)
    LAST_RESULTS = res
    out = np.concatenate([res.results[c]["out"] for c in range(_NCORES)], axis=0)
    return out
